# revision 3
# baseline (speedup 1.0000x reference)
"""Distributed Trainium2 kernel for a dense transformer block.

Sharding (8 cores, one chip):
  - LN1/LN2 + FFN: sequence-parallel (each core owns 512 of the 4096 tokens).
  - Attention: head-parallel (each core owns 2 of the 16 heads).
  - Collectives: AllGather of ln1(x)^T (feature-major), AllToAll of per-head
    attention outputs back to token shards.  No AllReduce needed.
  - Matmuls run in float32r (full-rate reduced-precision fp32) with fp32
    accumulation in PSUM.
"""

import sys

sys.path.insert(0, "/opt/trn_rl_repo")

import numpy as np

import concourse.bacc as bacc
import concourse.bass as bass
import concourse.tile as tile
from concourse import mybir
from concourse.masks import make_identity

F32 = mybir.dt.float32
F32R = mybir.dt.float32r
BF16 = mybir.dt.bfloat16
AF = mybir.ActivationFunctionType

N_CORES = 8
B, T, D, H = 2, 2048, 1024, 16
HD = D // H            # 64
NTOK = B * T           # 4096
S = NTOK // N_CORES    # 512 tokens per core
HPC = H // N_CORES     # 2 heads per core
E = HPC * HD           # 128 head-dim columns per core
F = 4 * D              # 4096 ffn hidden
EPS = 1e-5
SCALE = float(D) ** -0.5
MASK_VAL = -30000.0
P = 128

KT = D // P            # 8 feature tiles
TT = S // P            # 4 token tiles in the shard
NW = N_CORES           # 8 global 512-token windows
ST_B = T // P          # 16 s-tiles per batch
FT = F // P            # 32 ffn-hidden tiles

_CACHE = {}


def _build(n_chain=1, stub_cc=False, upto=9):
    nc = bacc.Bacc("TRN2", target_bir_lowering=False, debug=False,
                   num_devices=N_CORES)

    x = nc.dram_tensor("x", [S, D], F32, kind="ExternalInput")
    wq = nc.dram_tensor("wq", [D, E], BF16, kind="ExternalInput")
    wk = nc.dram_tensor("wk", [D, E], BF16, kind="ExternalInput")
    wv = nc.dram_tensor("wv", [D, E], BF16, kind="ExternalInput")
    wo = nc.dram_tensor("wo", [D, D], F32R, kind="ExternalInput")
    w1 = nc.dram_tensor("w1", [D, F], F32R, kind="ExternalInput")
    w2 = nc.dram_tensor("w2", [F, D], F32R, kind="ExternalInput")
    bo = nc.dram_tensor("bo", [D], F32, kind="ExternalInput")
    b1 = nc.dram_tensor("b1", [F], F32, kind="ExternalInput")
    b2 = nc.dram_tensor("b2", [D], F32, kind="ExternalInput")
    ln1_g = nc.dram_tensor("ln1_g", [D], F32, kind="ExternalInput")
    ln1_b = nc.dram_tensor("ln1_b", [D], F32, kind="ExternalInput")
    ln2_g = nc.dram_tensor("ln2_g", [D], F32, kind="ExternalInput")
    ln2_b = nc.dram_tensor("ln2_b", [D], F32, kind="ExternalInput")
    y = nc.dram_tensor("y", [S, D], F32, kind="ExternalOutput")
    global _W
    _W = dict(wq=wq, wk=wk, wv=wv, wo=wo, w1=w1, w2=w2, bo=bo, b1=b1, b2=b2,
              ln1_g=ln1_g, ln1_b=ln1_b, ln2_g=ln2_g, ln2_b=ln2_b)

    with tile.TileContext(nc) as tc:
      with tc.tile_pool(name="dram0", bufs=1, space="DRAM") as dram0:
        chain_bufs = [dram0.tile([S, D], F32, tag=f"chain{i}", name=f"chain{i}")
                      for i in range(n_chain - 1)]
        for _ci in range(n_chain):
            x_cur = x if _ci == 0 else chain_bufs[_ci - 1]
            y_cur = y if _ci == n_chain - 1 else chain_bufs[_ci]
            _emit_body(nc, tc, x_cur, y_cur, _ci, stub_cc, upto)

    nc.compile()
    return nc


def _emit_body(nc, tc, x, y, ci, stub_cc=False, upto=9):
    wq, wk, wv, wo = _W["wq"], _W["wk"], _W["wv"], _W["wo"]
    w1, w2, bo, b1, b2 = _W["w1"], _W["w2"], _W["bo"], _W["b1"], _W["b2"]
    ln1_g, ln1_b = _W["ln1_g"], _W["ln1_b"]
    ln2_g, ln2_b = _W["ln2_g"], _W["ln2_b"]
    with 1 == 1 and tc.tile_pool(name=f"body{ci}", bufs=1) as _unused:
        with tc.tile_pool(name="dram", bufs=1, space="DRAM") as dram, \
             tc.tile_pool(name="const", bufs=1) as const, \
             tc.tile_pool(name="persist", bufs=1) as persist:

            hT_sh = dram.tile([D, S], BF16, tag="hT_sh", name="hT_sh")
            hT_all = dram.tile([N_CORES * D, S], BF16, tag="hT_all",
                               name="hT_all", addr_space="Shared")
            a2a_in = dram.tile([NW * P, S], F32R, tag="a2a_in", name="a2a_in")
            a2a_out = dram.tile([NW * P, S], F32R, tag="a2a_out",
                                name="a2a_out")

            # ---- constants ----
            ident = const.tile([P, P], F32, tag="ident", name="ident")
            make_identity(nc, ident)

            ones_r = const.tile([P, HD], BF16, tag="ones_r", name="ones_r")
            nc.vector.memset(ones_r[:], 1.0)

            eps_t = const.tile([P, 1], F32, tag="eps", name="eps_t")
            nc.vector.memset(eps_t[:], EPS)

            # ln params, feature-major [128, KT]
            g1_s = const.tile([P, KT], F32, tag="g1", name="g1_s")
            b1l_s = const.tile([P, KT], F32, tag="b1l", name="b1l_s")
            g2_s = const.tile([P, KT], F32, tag="g2", name="g2_s")
            b2l_s = const.tile([P, KT], F32, tag="b2l", name="b2l_s")
            nc.sync.dma_start(out=g1_s[:],
                              in_=ln1_g.ap().rearrange("(k p) -> p k", p=P))
            nc.sync.dma_start(out=b1l_s[:],
                              in_=ln1_b.ap().rearrange("(k p) -> p k", p=P))
            nc.sync.dma_start(out=g2_s[:],
                              in_=ln2_g.ap().rearrange("(k p) -> p k", p=P))
            nc.sync.dma_start(out=b2l_s[:],
                              in_=ln2_b.ap().rearrange("(k p) -> p k", p=P))

            # b1 (ffn bias), feature-major [128, FT]
            b1_s = const.tile([P, FT], F32, tag="b1s", name="b1_s")
            nc.sync.dma_start(out=b1_s[:],
                              in_=b1.ap().rearrange("(k p) -> p k", p=P))

            # bo, b2 broadcast across partitions [128, D]
            bo_bc = const.tile([P, D], F32, tag="bo_bc", name="bo_bc")
            b2_bc = const.tile([P, D], F32, tag="b2_bc", name="b2_bc")
            nc.sync.dma_start(out=bo_bc[:], in_=bo.ap().partition_broadcast(P))
            nc.sync.dma_start(out=b2_bc[:], in_=b2.ap().partition_broadcast(P))

            # persistent across most of the kernel: x shard, r1, h2T
            x_sb = [persist.tile([P, D], F32, tag=f"x{i}", name=f"x{i}")
                    for i in range(TT)]
            for i in range(TT):
                nc.sync.dma_start(out=x_sb[i][:], in_=x[i * P:(i + 1) * P, :])
            r1 = [persist.tile([P, D], F32, tag=f"r1_{i}", name=f"r1_{i}")
                  for i in range(TT)]
            h2T = [persist.tile([P, S], F32R, tag=f"h2T{k}", name=f"h2T{k}")
                   for k in range(KT)]

            def layernorm_tiles(src_tiles, pool, out_name):
                """LN over the free axis -> normalized [t,d] f32 tiles.
                gamma/beta are folded in at transpose-evict time."""
                out = []
                with tc.tile_pool(name=f"ln_{out_name}", bufs=2) as lnp:
                    for i, xt in enumerate(src_tiles):
                        st = lnp.tile([P, 2, 6], F32, tag="stats", name="st")
                        xr = xt[:].rearrange("p (s f) -> p s f", s=2)
                        for sg in range(2):
                            nc.vector.bn_stats(out=st[:, sg, :], in_=xr[:, sg, :])
                        mv = lnp.tile([P, 2], F32, tag="mv", name="mv")
                        nc.vector.bn_aggr(out=mv[:], in_=st[:])
                        rstd = lnp.tile([P, 1], F32, tag="rstd", name="rstd")
                        nc.scalar.activation(out=rstd[:], in_=mv[:, 1:2],
                                             func=AF.Sqrt, bias=eps_t[:],
                                             scale=1.0)
                        nc.vector.reciprocal(out=rstd[:], in_=rstd[:])
                        o = pool.tile([P, D], F32, tag=f"{out_name}{i}",
                                      name=f"{out_name}{i}")
                        nc.vector.tensor_scalar(
                            out=o[:], in0=xt[:],
                            scalar1=mv[:, 0:1], scalar2=rstd[:],
                            op0=mybir.AluOpType.subtract,
                            op1=mybir.AluOpType.mult,
                        )
                        out.append(o)
                return out

            # ============ attention super-phase (scoped SBUF) ============
            with tc.tile_pool(name="attnsb", bufs=1) as attnsb:
                # -------- LN1 + transpose + AllGather --------
                with nc.named_scope(f"ln1_{ci}"), \
                     tc.tile_pool(name="xlnp", bufs=1) as xlnp:
                    xln = layernorm_tiles(x_sb, xlnp, "xln")
                    with tc.tile_pool(name="tr1", bufs=4) as trp, \
                         tc.tile_pool(name="tr1p", bufs=4, space="PSUM") as trpp:
                        for kt in range(KT):
                            hb = trp.tile([P, S], BF16, tag="hb", name="hb")
                            for i in range(TT):
                                pt = trpp.tile([P, P], F32, tag="tr", name="pt")
                                nc.tensor.transpose(
                                    pt[:], xln[i][:, kt * P:(kt + 1) * P],
                                    ident[:])
                                nc.vector.tensor_scalar(
                                    out=hb[:, i * P:(i + 1) * P], in0=pt[:],
                                    scalar1=g1_s[:, kt:kt + 1],
                                    scalar2=b1l_s[:, kt:kt + 1],
                                    op0=mybir.AluOpType.mult,
                                    op1=mybir.AluOpType.add,
                                )
                            nc.sync.dma_start(
                                out=hT_sh[kt * P:(kt + 1) * P, :], in_=hb[:])

                with nc.named_scope(f"allgather_{ci}"):
                  if stub_cc:
                    nc.sync.dma_start(out=hT_all[0:D, :], in_=hT_sh[:, :])
                  else:
                    nc.gpsimd.collective_compute(
                        "AllGather", mybir.AluOpType.bypass,
                        replica_groups=[list(range(N_CORES))],
                        ins=[hT_sh.opt()], outs=[hT_all.opt()],
                    )

                # -------- QKV projections --------
                if upto < 2:
                    return
                _qkv_scope = nc.named_scope(f"qkv_{ci}"); _qkv_scope.__enter__()
                wq_sb = [attnsb.tile([P, E], BF16, tag=f"wq{k}", name=f"wq{k}")
                         for k in range(KT)]
                wk_sb = [attnsb.tile([P, E], BF16, tag=f"wk{k}", name=f"wk{k}")
                         for k in range(KT)]
                wv_sb = [attnsb.tile([P, E], BF16, tag=f"wv{k}", name=f"wv{k}")
                         for k in range(KT)]
                for k in range(KT):
                    nc.sync.dma_start(out=wq_sb[k][:],
                                      in_=wq[k * P:(k + 1) * P, :])
                    nc.sync.dma_start(out=wk_sb[k][:],
                                      in_=wk[k * P:(k + 1) * P, :])
                    nc.sync.dma_start(out=wv_sb[k][:],
                                      in_=wv[k * P:(k + 1) * P, :])

                qT_w = [attnsb.tile([P, 512], BF16, tag=f"qT{w}",
                                    name=f"qT{w}") for w in range(NW)]
                kT_w = [attnsb.tile([P, 512], BF16, tag=f"kT{w}",
                                    name=f"kT{w}") for w in range(NW)]
                v_sb = [attnsb.tile([P, E], BF16, tag=f"v{s}", name=f"v{s}")
                        for s in range(NTOK // P)]

                with tc.tile_pool(name="hstream", bufs=12) as hsp, \
                     tc.tile_pool(name="vtmp", bufs=2) as vtp, \
                     tc.tile_pool(name="qkvp", bufs=2, space="PSUM") as qkvp, \
                     tc.tile_pool(name="vtrp", bufs=2, space="PSUM") as vtrp:
                    for tch in range(NW):
                        psq = qkvp.tile([P, 512], F32, tag="psq", name="psq")
                        psk = qkvp.tile([P, 512], F32, tag="psk", name="psk")
                        psv = qkvp.tile([P, 512], F32, tag="psv", name="psv")
                        for kt in range(KT):
                            ht = hsp.tile([P, 512], BF16, tag="ht", name="ht")
                            nc.sync.dma_start(
                                out=ht[:],
                                in_=hT_all[tch * D + kt * P:
                                           tch * D + (kt + 1) * P, :])
                            first, last = kt == 0, kt == KT - 1
                            nc.tensor.matmul(psq[:], wq_sb[kt][:], ht[:],
                                             start=first, stop=last)
                            nc.tensor.matmul(psk[:], wk_sb[kt][:], ht[:],
                                             start=first, stop=last)
                            nc.tensor.matmul(psv[:], wv_sb[kt][:], ht[:],
                                             start=first, stop=last)
                        nc.vector.tensor_copy(qT_w[tch][:], psq[:])
                        nc.vector.tensor_copy(kT_w[tch][:], psk[:])
                        vt = vtp.tile([P, 512], F32, tag="vt", name="vt")
                        nc.scalar.copy(vt[:], psv[:])
                        for j in range(4):
                            pv = vtrp.tile([P, P], F32, tag="pv", name="pv")
                            nc.tensor.transpose(pv[:], vt[:, j * P:(j + 1) * P],
                                                ident[:])
                            nc.vector.tensor_copy(v_sb[tch * 4 + j][:], pv[:])

                _qkv_scope.__exit__(None, None, None)
                # -------- attention --------
                if upto < 3:
                    return
                _att_scope = nc.named_scope(f"attn_{ci}"); _att_scope.__enter__()
                masks = []
                for k in range(4):
                    m = attnsb.tile([P, 512], F32, tag=f"mask{k}",
                                    name=f"mask{k}")
                    nc.gpsimd.memset(m[:], 0.0)
                    nc.gpsimd.affine_select(
                        out=m[:], in_=m[:],
                        compare_op=mybir.AluOpType.is_ge,
                        fill=MASK_VAL, base=-128 * k,
                        pattern=[[1, 512]], channel_multiplier=-1,
                    )
                    masks.append(m)
                with tc.tile_pool(name="pt_pool", bufs=4) as ptp, \
                     tc.tile_pool(name="attno", bufs=6) as aop, \
                     tc.tile_pool(name="scp", bufs=2, space="PSUM") as scp, \
                     tc.tile_pool(name="lop", bufs=1, space="PSUM") as lop:
                    for b in range(B):
                        for tcl in range(T // 512):
                            tch = b * (T // 512) + tcl
                            l_psa = lop.tile([HD, 512], F32, tag="la", name="l_psa")
                            l_psb = lop.tile([HD, 512], F32, tag="lb", name="l_psb")
                            o_psa = lop.tile([HD, 512], F32, tag="oa", name="o_psa")
                            o_psb = lop.tile([HD, 512], F32, tag="ob", name="o_psb")
                            n_s = 4 * (tcl + 1)
                            for si in range(n_s):
                                sg = b * ST_B + si
                                sc_a = scp.tile([P, 512], F32, tag="sca",
                                                name="sc_a")
                                sc_b = scp.tile([P, 512], F32, tag="scb",
                                                name="sc_b")
                                kw = kT_w[sg // 4]
                                kc = (sg % 4) * P
                                nc.tensor.matmul(
                                    sc_a[:], kw[0:HD, kc:kc + P],
                                    qT_w[tch][0:HD, :],
                                    start=True, stop=True,
                                    tile_position=(0, 0))
                                nc.tensor.matmul(
                                    sc_b[:], kw[HD:P, kc:kc + P],
                                    qT_w[tch][HD:P, :],
                                    start=True, stop=True,
                                    tile_position=(64, 0))
                                p_a = ptp.tile([P, 512], BF16, tag="pa",
                                               name="p_a")
                                p_b = ptp.tile([P, 512], BF16, tag="pb",
                                               name="p_b")
                                if si // 4 == tcl:
                                    c0 = (si % 4) * P
                                    mk = masks[si % 4]
                                    for sc_, p_ in ((sc_a, p_a), (sc_b, p_b)):
                                        nc.vector.tensor_add(
                                            out=sc_[:, c0:], in0=sc_[:, c0:],
                                            in1=mk[:, c0:])
                                        if c0:
                                            nc.vector.memset(p_[:, 0:c0], 0.0)
                                        nc.scalar.activation(
                                            out=p_[:, c0:], in_=sc_[:, c0:],
                                            func=AF.Exp, scale=SCALE)
                                else:
                                    nc.scalar.activation(out=p_a[:],
                                                         in_=sc_a[:],
                                                         func=AF.Exp,
                                                         scale=SCALE)
                                    nc.scalar.activation(out=p_b[:],
                                                         in_=sc_b[:],
                                                         func=AF.Exp,
                                                         scale=SCALE)
                                first, last = si == 0, si == n_s - 1
                                nc.tensor.matmul(l_psa[:], ones_r[:, 0:HD],
                                                 p_a[:], start=first, stop=last)
                                nc.tensor.matmul(l_psb[:], ones_r[:, 0:HD],
                                                 p_b[:], start=first, stop=last)
                                nc.tensor.matmul(o_psa[:],
                                                 v_sb[sg][:, 0:HD], p_a[:],
                                                 start=first, stop=last)
                                nc.tensor.matmul(o_psb[:],
                                                 v_sb[sg][:, HD:E], p_b[:],
                                                 start=first, stop=last)
                            linv = aop.tile([P, 512], F32, tag="linv",
                                            name="linv")
                            nc.vector.reciprocal(out=linv[0:HD, :], in_=l_psa[:])
                            nc.vector.reciprocal(out=linv[HD:P, :], in_=l_psb[:])
                            o_n = aop.tile([P, 512], F32R, tag="on", name="o_n")
                            nc.vector.tensor_mul(out=o_n[0:HD, :], in0=o_psa[:],
                                                 in1=linv[0:HD, :])
                            nc.vector.tensor_mul(out=o_n[HD:P, :], in0=o_psb[:],
                                                 in1=linv[HD:P, :])
                            nc.sync.dma_start(
                                out=a2a_in[tch * P:(tch + 1) * P, :],
                                in_=o_n[:])

                _att_scope.__exit__(None, None, None)
                _a2a_scope = nc.named_scope(f"a2a_{ci}"); _a2a_scope.__enter__()
                if stub_cc:
                    nc.sync.dma_start(out=a2a_out[:, :], in_=a2a_in[:, :])
                else:
                    nc.gpsimd.collective_compute(
                        "AllToAll", mybir.AluOpType.bypass,
                        replica_groups=[list(range(N_CORES))],
                        ins=[a2a_in.opt()], outs=[a2a_out.opt()],
                    )
                _a2a_scope.__exit__(None, None, None)
            # attnsb closed: qT/kT/v/wqkv SBUF freed

            # -------- output projection + residual --------
            if upto < 4:
                return
            with nc.named_scope(f"wo_{ci}"), \
                 tc.tile_pool(name="wos", bufs=3) as wos, \
                 tc.tile_pool(name="aos", bufs=6) as aos, \
                 tc.tile_pool(name="wop", bufs=1, space="PSUM") as wop:
                pso = [wop.tile([P, 512], F32, tag=f"wo{i}", name=f"wo{i}")
                       for i in range(8)]
                for kt in range(KT):
                    ao = aos.tile([P, S], F32R, tag="ao", name="ao")
                    nc.sync.dma_start(out=ao[:],
                                      in_=a2a_out[kt * P:(kt + 1) * P, :])
                    wot = wos.tile([P, D], F32R, tag="wot", name="wot")
                    nc.sync.dma_start(out=wot[:],
                                      in_=wo[kt * P:(kt + 1) * P, :])
                    first, last = kt == 0, kt == KT - 1
                    for tt in range(TT):
                        for dc in range(2):
                            nc.tensor.matmul(
                                pso[tt * 2 + dc][:],
                                ao[:, tt * P:(tt + 1) * P],
                                wot[:, dc * 512:(dc + 1) * 512],
                                start=first, stop=last)
                for tt in range(TT):
                    for dc in range(2):
                        sl = slice(dc * 512, (dc + 1) * 512)
                        nc.vector.tensor_add(out=r1[tt][:, sl],
                                             in0=pso[tt * 2 + dc][:],
                                             in1=x_sb[tt][:, sl])
                        nc.vector.tensor_add(out=r1[tt][:, sl],
                                             in0=r1[tt][:, sl],
                                             in1=bo_bc[:, sl])

            # -------- LN2 + transpose --------
            if upto < 5:
                return
            with nc.named_scope(f"ln2_{ci}"), \
                 tc.tile_pool(name="h2p", bufs=1) as h2p:
                h2 = layernorm_tiles(r1, h2p, "h2")
                with tc.tile_pool(name="tr2p", bufs=4, space="PSUM") as tr2p:
                    for i in range(TT):
                        for kt in range(KT):
                            pt2 = tr2p.tile([P, P], F32, tag="tr2", name="pt2")
                            nc.tensor.transpose(
                                pt2[:], h2[i][:, kt * P:(kt + 1) * P], ident[:])
                            nc.vector.tensor_scalar(
                                out=h2T[kt][:, i * P:(i + 1) * P], in0=pt2[:],
                                scalar1=g2_s[:, kt:kt + 1],
                                scalar2=b2l_s[:, kt:kt + 1],
                                op0=mybir.AluOpType.mult,
                                op1=mybir.AluOpType.add,
                            )

            # -------- FFN --------
            if upto < 6:
                return
            with tc.tile_pool(name="ff1sb", bufs=1) as ff1sb:
                _ff1_scope = nc.named_scope(f"ffn1_{ci}"); _ff1_scope.__enter__()
                ff1 = [ff1sb.tile([P, S], F32R, tag=f"ff1_{k}",
                                  name=f"ff1_{k}") for k in range(FT)]
                with tc.tile_pool(name="w1s", bufs=1) as w1s, \
                     tc.tile_pool(name="ff1p", bufs=8, space="PSUM") as ff1p:
                    FH = F // 2
                    for half in range(2):
                        w1h = [w1s.tile([P, FH], F32R, tag=f"w1h{k}",
                                        name=f"w1h{k}") for k in range(KT)]
                        for k in range(KT):
                            nc.sync.dma_start(
                                out=w1h[k][:],
                                in_=w1[k * P:(k + 1) * P,
                                       half * FH:(half + 1) * FH])
                        for fl in range(FH // P):
                            ft = half * (FH // P) + fl
                            ps = ff1p.tile([P, S], F32, tag="ff1", name="ps")
                            for kt in range(KT):
                                nc.tensor.matmul(
                                    ps[:], w1h[kt][:, fl * P:(fl + 1) * P],
                                    h2T[kt][:],
                                    start=(kt == 0), stop=(kt == KT - 1))
                            nc.scalar.activation(out=ff1[ft][:], in_=ps[:],
                                                 func=AF.Relu,
                                                 bias=b1_s[:, ft:ft + 1])

                _ff1_scope.__exit__(None, None, None)
                if upto < 7:
                    return
                with nc.named_scope(f"ffn2_{ci}"), \
                     tc.tile_pool(name="w2s", bufs=4) as w2s, \
                     tc.tile_pool(name="outp", bufs=4) as outp, \
                     tc.tile_pool(name="ff2p", bufs=1, space="PSUM") as ff2p:
                    ps2 = [ff2p.tile([P, 512], F32, tag=f"ff2_{i}",
                                     name=f"ff2_{i}") for i in range(8)]
                    for kt in range(FT):
                        w2t = w2s.tile([P, D], F32R, tag="w2t", name="w2t")
                        nc.sync.dma_start(out=w2t[:],
                                          in_=w2[kt * P:(kt + 1) * P, :])
                        first, last = kt == 0, kt == FT - 1
                        for tt in range(TT):
                            for dc in range(2):
                                nc.tensor.matmul(
                                    ps2[tt * 2 + dc][:],
                                    ff1[kt][:, tt * P:(tt + 1) * P],
                                    w2t[:, dc * 512:(dc + 1) * 512],
                                    start=first, stop=last)
                    for tt in range(TT):
                        for dc in range(2):
                            sl = slice(dc * 512, (dc + 1) * 512)
                            ot = outp.tile([P, 512], F32, tag="ot", name="ot")
                            nc.vector.tensor_add(out=ot[:],
                                                 in0=ps2[tt * 2 + dc][:],
                                                 in1=r1[tt][:, sl])
                            nc.vector.tensor_add(out=ot[:], in0=ot[:],
                                                 in1=b2_bc[:, sl])
                            nc.sync.dma_start(out=y[tt * P:(tt + 1) * P, sl],
                                              in_=ot[:])


def _shard_inputs(inputs):
    x = np.ascontiguousarray(np.asarray(inputs["x"], np.float32).reshape(NTOK, D))
    Wq = np.asarray(inputs["Wq"], np.float32)
    Wk = np.asarray(inputs["Wk"], np.float32)
    Wv = np.asarray(inputs["Wv"], np.float32)
    com = dict(
        wo=np.ascontiguousarray(np.asarray(inputs["Wo"], np.float32)),
        w1=np.ascontiguousarray(np.asarray(inputs["W1"], np.float32)),
        w2=np.ascontiguousarray(np.asarray(inputs["W2"], np.float32)),
        bo=np.asarray(inputs["bo"], np.float32),
        b1=np.asarray(inputs["b1"], np.float32),
        b2=np.asarray(inputs["b2"], np.float32),
        ln1_g=np.asarray(inputs["ln1_g"], np.float32),
        ln1_b=np.asarray(inputs["ln1_b"], np.float32),
        ln2_g=np.asarray(inputs["ln2_g"], np.float32),
        ln2_b=np.asarray(inputs["ln2_b"], np.float32),
    )
    maps = []
    for c in range(N_CORES):
        hs = slice(HPC * c, HPC * (c + 1))
        m = dict(com)
        m["x"] = x[c * S:(c + 1) * S]
        import ml_dtypes
        bf = ml_dtypes.bfloat16
        m["wq"] = np.ascontiguousarray(
            Wq[hs].transpose(1, 0, 2).reshape(D, E).astype(bf))
        m["wk"] = np.ascontiguousarray(
            Wk[hs].transpose(1, 0, 2).reshape(D, E).astype(bf))
        m["wv"] = np.ascontiguousarray(
            Wv[hs].transpose(1, 0, 2).reshape(D, E).astype(bf))
        maps.append(m)
    return maps


def _get_nc():
    if "nc" not in _CACHE:
        _CACHE["nc"] = _build()
    return _CACHE["nc"]


def _run(in_maps):
    from concourse.bass_utils import run_bass_kernel_spmd
    nc = _get_nc()
    res = run_bass_kernel_spmd(nc, in_maps, core_ids=list(range(N_CORES)))
    return res.results


def kernel(**inputs):
    in_maps = _shard_inputs(inputs)
    results = _run(in_maps)
    out = np.concatenate([results[c]["y"] for c in range(N_CORES)], axis=0)
    return out.reshape(B, T, D)



# revision 13
# speedup vs baseline: 1.0395x; 1.0395x over previous
"""Distributed Trainium2 kernel for a dense transformer block.

Sharding (8 cores, one chip):
  - LN1/LN2 + FFN + Wo: sequence-parallel (each core owns 512 of 4096 tokens).
  - Attention: head-parallel (each core owns 2 of the 16 heads).
  - Collectives: one fused AllToAll of q/k/v (token-shard -> head-shard) and
    one AllToAll of per-head attention outputs back to token shards.
  - Softmax denominator comes free from a ones-column appended to V; the
    per-head normalizer is broadcast across partitions with a tiny select
    matmul.  exp() runs once per key-block over both heads [128, 1024].
  - All weights and inter-core payloads in bf16 (fp32 accumulate in PSUM);
    x / residual / output stay fp32.
"""

import sys

sys.path.insert(0, "/opt/trn_rl_repo")

import numpy as np

import concourse.bacc as bacc
import concourse.bass as bass
import concourse.tile as tile
from concourse import mybir
from concourse.masks import make_identity

F32 = mybir.dt.float32
F32R = mybir.dt.float32r
BF16 = mybir.dt.bfloat16
AF = mybir.ActivationFunctionType
ALU = mybir.AluOpType

N_CORES = 8
B, T, D, H = 2, 2048, 1024, 16
HD = D // H            # 64
NTOK = B * T           # 4096
S = NTOK // N_CORES    # 512 tokens per core
F = 4 * D              # 4096 ffn hidden
EPS = 1e-5
SCALE = float(D) ** -0.5
MASK_VAL = -30000.0
P = 128

KT = D // P            # 8 feature tiles
TT = S // P            # 4 token tiles in the shard
NW = N_CORES           # 8 global 512-token windows
FT = F // P            # 32 ffn-hidden tiles

_CACHE = {}
_W = {}


def _build(n_chain=1, stub_cc=False):
    nc = bacc.Bacc("TRN2", target_bir_lowering=False, debug=False,
                   num_devices=N_CORES)

    x = nc.dram_tensor("x", [S, D], F32, kind="ExternalInput")
    wq = nc.dram_tensor("wq", [D, D], BF16, kind="ExternalInput")
    wk = nc.dram_tensor("wk", [D, D], BF16, kind="ExternalInput")
    wv = nc.dram_tensor("wv", [D, D], BF16, kind="ExternalInput")
    wo = nc.dram_tensor("wo", [D, D], BF16, kind="ExternalInput")
    w1 = nc.dram_tensor("w1", [D, F], BF16, kind="ExternalInput")
    w2 = nc.dram_tensor("w2", [F, D], BF16, kind="ExternalInput")
    bo = nc.dram_tensor("bo", [D], BF16, kind="ExternalInput")
    b1 = nc.dram_tensor("b1", [F], F32, kind="ExternalInput")
    b2 = nc.dram_tensor("b2", [D], BF16, kind="ExternalInput")
    ln1_g = nc.dram_tensor("ln1_g", [D], F32, kind="ExternalInput")
    ln1_b = nc.dram_tensor("ln1_b", [D], F32, kind="ExternalInput")
    ln2_g = nc.dram_tensor("ln2_g", [D], F32, kind="ExternalInput")
    ln2_b = nc.dram_tensor("ln2_b", [D], F32, kind="ExternalInput")
    y = nc.dram_tensor("y", [S, D], F32, kind="ExternalOutput")
    global _W
    _W = dict(wq=wq, wk=wk, wv=wv, wo=wo, w1=w1, w2=w2, bo=bo, b1=b1, b2=b2,
              ln1_g=ln1_g, ln1_b=ln1_b, ln2_g=ln2_g, ln2_b=ln2_b)

    with tile.TileContext(nc) as tc:
      with tc.tile_pool(name="dram0", bufs=1, space="DRAM") as dram0, \
           tc.tile_pool(name="const", bufs=1) as const:
        # ------- constants (once for the whole chain) -------
        C = {}
        C["ident"] = const.tile([P, P], BF16, tag="ident", name="ident")
        make_identity(nc, C["ident"][:])

        C["eps"] = const.tile([P, 1], F32, tag="eps", name="eps")
        nc.vector.memset(C["eps"][:], EPS)

        for nm, src in (("g1", ln1_g), ("b1l", ln1_b),
                        ("g2", ln2_g), ("b2l", ln2_b)):
            t = const.tile([P, KT], F32, tag=nm, name=nm)
            nc.sync.dma_start(out=t[:],
                              in_=src.ap().rearrange("(k p) -> p k", p=P))
            C[nm] = t

        C["b1s"] = const.tile([P, FT], F32, tag="b1s", name="b1s")
        nc.sync.dma_start(out=C["b1s"][:],
                          in_=b1.ap().rearrange("(k p) -> p k", p=P))

        C["bor"] = const.tile([1, D], BF16, tag="bor", name="bor")
        nc.sync.dma_start(out=C["bor"][:],
                          in_=bo.ap().rearrange("(a d) -> a d", a=1))
        C["b2r"] = const.tile([1, D], BF16, tag="b2r", name="b2r")
        nc.sync.dma_start(out=C["b2r"][:],
                          in_=b2.ap().rearrange("(a d) -> a d", a=1))

        C["ones1"] = const.tile([1, P], BF16, tag="ones1", name="ones1")
        nc.vector.memset(C["ones1"][:], 1.0)

        selA = const.tile([1, P], BF16, tag="selA", name="selA")
        nc.vector.memset(selA[:], 0.0)
        nc.vector.memset(selA[0:1, 0:HD], 1.0)
        C["selA"] = selA
        selB = const.tile([1, P], BF16, tag="selB", name="selB")
        nc.vector.memset(selB[:], 0.0)
        nc.vector.memset(selB[0:1, HD:P], 1.0)
        C["selB"] = selB

        masks = []
        for k in range(4):
            m = const.tile([P, S], F32, tag=f"mask{k}", name=f"mask{k}")
            nc.gpsimd.memset(m[:], 0.0)
            nc.gpsimd.affine_select(
                out=m[:], in_=m[:],
                compare_op=ALU.is_ge,
                fill=MASK_VAL, base=-P * k,
                pattern=[[1, S]], channel_multiplier=-1,
            )
            masks.append(m)
        C["masks"] = masks

        chain_bufs = [dram0.tile([S, D], F32, tag=f"chain{i}",
                                 name=f"chain{i}")
                      for i in range(n_chain - 1)]
        for _ci in range(n_chain):
            x_cur = x if _ci == 0 else chain_bufs[_ci - 1]
            y_cur = y if _ci == n_chain - 1 else chain_bufs[_ci]
            _emit_body(nc, tc, C, x_cur, y_cur, _ci, stub_cc)

    nc.compile()
    return nc


def _layernorm(nc, tc, C, src_big, pool, out_name):
    """LN over the free axis of 4 [P, D] column-slices of src_big.

    Returns 4 normalized bf16 [P, D] tiles (gamma/beta folded in later at
    transpose-evict time)."""
    out = []
    with tc.tile_pool(name=f"ln_{out_name}", bufs=2) as lnp:
        for i in range(TT):
            sl = src_big[:, i * D:(i + 1) * D]
            st = lnp.tile([P, 2, 6], F32, tag="stats", name="st")
            xr = sl.rearrange("p (s f) -> p s f", s=2)
            for sg in range(2):
                nc.vector.bn_stats(out=st[:, sg, :], in_=xr[:, sg, :])
            mv = lnp.tile([P, 2], F32, tag="mv", name="mv")
            nc.vector.bn_aggr(out=mv[:], in_=st[:])
            rstd = lnp.tile([P, 1], F32, tag="rstd", name="rstd")
            nc.scalar.activation(out=rstd[:], in_=mv[:, 1:2],
                                 func=AF.Sqrt, bias=C["eps"][:], scale=1.0)
            nc.vector.reciprocal(out=rstd[:], in_=rstd[:])
            o = pool.tile([P, D], BF16, tag=f"{out_name}{i}",
                          name=f"{out_name}{i}")
            nc.vector.tensor_scalar(
                out=o[:], in0=sl,
                scalar1=mv[:, 0:1], scalar2=rstd[:],
                op0=ALU.subtract, op1=ALU.mult,
            )
            out.append(o)
    return out


def _emit_body(nc, tc, C, x, y, ci, stub_cc=False):
    wq, wk, wv, wo = _W["wq"], _W["wk"], _W["wv"], _W["wo"]
    w1, w2 = _W["w1"], _W["w2"]
    ident = C["ident"]
    masks = C["masks"]

    with tc.tile_pool(name=f"dram{ci}", bufs=1, space="DRAM") as dram, \
         tc.tile_pool(name=f"persist{ci}", bufs=1) as persist:

        a2aq_in = dram.tile([3 * NW * P, S], BF16, tag="a2aq_in",
                            name="a2aq_in")
        a2aq_out = dram.tile([3 * NW * P, S], BF16, tag="a2aq_out",
                             name="a2aq_out")
        a2ao_in = dram.tile([NW * P, S], BF16, tag="a2ao_in", name="a2ao_in")
        a2ao_out = dram.tile([NW * P, S], BF16, tag="a2ao_out",
                             name="a2ao_out")

        x_big = persist.tile([P, TT * D], F32, tag="x_big", name="x_big")
        nc.sync.dma_start(
            out=x_big[:].rearrange("p (t d) -> p t d", t=TT),
            in_=x[:, :].rearrange("(t p) d -> p t d", p=P))
        r1_big = persist.tile([P, TT * D], F32, tag="r1_big", name="r1_big")

        # ================= LN1 + QKV (token-parallel) =================
        with nc.named_scope(f"qkv_{ci}"), \
             tc.tile_pool(name="wqkv", bufs=1) as wqkvp, \
             tc.tile_pool(name="hbp", bufs=1) as hbp:
            wq_sb = wqkvp.tile([P, KT * D], BF16, tag="wq", name="wq_sb")
            wk_sb = wqkvp.tile([P, KT * D], BF16, tag="wk", name="wk_sb")
            wv_sb = wqkvp.tile([P, KT * D], BF16, tag="wv", name="wv_sb")
            for t, src in ((wq_sb, wq), (wk_sb, wk), (wv_sb, wv)):
                nc.scalar.dma_start(
                    out=t[:].rearrange("p (k e) -> p k e", k=KT),
                    in_=src.ap().rearrange("(k p) e -> p k e", p=P))

            with tc.tile_pool(name="xlnp", bufs=1) as xlnp:
                xln = _layernorm(nc, tc, C, x_big[:], xlnp, f"xln{ci}")
                hb = [hbp.tile([P, S], BF16, tag=f"hb{k}", name=f"hb{k}")
                      for k in range(KT)]
                with tc.tile_pool(name="tr1p", bufs=4, space="PSUM") as trpp:
                    for kt in range(KT):
                        for i in range(TT):
                            pt = trpp.tile([P, P], BF16, tag="tr", name="pt")
                            nc.tensor.transpose(
                                pt[:], xln[i][:, kt * P:(kt + 1) * P],
                                ident[:])
                            nc.vector.tensor_scalar(
                                out=hb[kt][:, i * P:(i + 1) * P], in0=pt[:],
                                scalar1=C["g1"][:, kt:kt + 1],
                                scalar2=C["b1l"][:, kt:kt + 1],
                                op0=ALU.mult, op1=ALU.add,
                            )

            # local QKV for own tokens, all 16 heads
            with tc.tile_pool(name="qkvps", bufs=2, space="PSUM") as qkvps, \
                 tc.tile_pool(name="stgo", bufs=2) as stgo:
                for hp in range(NW):
                    psq = qkvps.tile([P, S], F32, tag="psq", name="psq")
                    psk = qkvps.tile([P, S], F32, tag="psk", name="psk")
                    psv = qkvps.tile([P, S], F32, tag="psv", name="psv")
                    for kt in range(KT):
                        first, last = kt == 0, kt == KT - 1
                        c = kt * D + hp * P
                        nc.tensor.matmul(psq[:], wq_sb[:, c:c + P],
                                         hb[kt][:], start=first, stop=last)
                        nc.tensor.matmul(psk[:], wk_sb[:, c:c + P],
                                         hb[kt][:], start=first, stop=last)
                        nc.tensor.matmul(psv[:], wv_sb[:, c:c + P],
                                         hb[kt][:], start=first, stop=last)
                    stg = stgo.tile([P, 3 * S], BF16, tag="stg", name="stg")
                    nc.vector.tensor_copy(stg[:, 0:S], psq[:])
                    nc.vector.tensor_copy(stg[:, S:2 * S], psk[:])
                    nc.vector.tensor_copy(stg[:, 2 * S:3 * S], psv[:])
                    nc.sync.dma_start(
                        out=a2aq_in[hp * 3 * P:(hp + 1) * 3 * P, :]
                            .rearrange("(c p) s -> p c s", p=P),
                        in_=stg[:].rearrange("p (c s) -> p c s", c=3))

        with nc.named_scope(f"a2aq_{ci}"):
            if stub_cc:
                nc.sync.dma_start(out=a2aq_out[:, :], in_=a2aq_in[:, :])
            else:
                nc.gpsimd.collective_compute(
                    "AllToAll", ALU.bypass,
                    replica_groups=[list(range(N_CORES))],
                    ins=[a2aq_in.opt()], outs=[a2aq_out.opt()],
                )

        # ================= attention (head-parallel) =================
        with nc.named_scope(f"attn_{ci}"), \
             tc.tile_pool(name="attnsb", bufs=1) as attnsb:
            qst = [attnsb.tile([P, 3 * S], BF16, tag=f"qst{w}",
                               name=f"qst{w}") for w in range(NW)]
            for w in range(NW):
                nc.sync.dma_start(
                    out=qst[w][:].rearrange("p (c s) -> p c s", c=3),
                    in_=a2aq_out[w * 3 * P:(w + 1) * 3 * P, :]
                        .rearrange("(c p) s -> p c s", p=P))

            # V with a ones-column appended per head: [tok, 65+65]
            v65 = [attnsb.tile([P, 130], BF16, tag=f"v65_{b}",
                               name=f"v65_{b}") for b in range(NTOK // P)]
            with tc.tile_pool(name="vtrp", bufs=2, space="PSUM") as vtrp:
                for w in range(NW):
                    for jb in range(TT):
                        pv = vtrp.tile([P, P], BF16, tag="pv", name="pv")
                        nc.tensor.transpose(
                            pv[:],
                            qst[w][:, 2 * S + jb * P:2 * S + (jb + 1) * P],
                            ident[:])
                        blk = w * TT + jb
                        nc.vector.tensor_copy(v65[blk][:, 0:HD],
                                              pv[:, 0:HD])
                        nc.vector.tensor_copy(v65[blk][:, 65:65 + HD],
                                              pv[:, HD:P])
                        nc.gpsimd.memset(v65[blk][:, HD:HD + 1], 1.0)
                        nc.gpsimd.memset(v65[blk][:, 65 + HD:66 + HD], 1.0)

            with tc.tile_pool(name="scp", bufs=2, space="PSUM") as scp, \
                 tc.tile_pool(name="pp", bufs=3) as pp, \
                 tc.tile_pool(name="lop", bufs=1, space="PSUM") as lop, \
                 tc.tile_pool(name="lbcp", bufs=1, space="PSUM") as lbcp, \
                 tc.tile_pool(name="onp", bufs=2) as onp, \
                 tc.tile_pool(name="l2p", bufs=2) as l2p:
                for b in range(B):
                    for tcl in range(T // S):
                        tch = b * (T // S) + tcl
                        o_psa = lop.tile([HD + 1, S], F32, tag="oa",
                                         name="o_psa")
                        o_psb = lop.tile([HD + 1, S], F32, tag="ob",
                                         name="o_psb")
                        n_s = TT * (tcl + 1)
                        for si in range(n_s):
                            sg = b * (T // P) + si
                            kst = qst[sg // TT]
                            kc = S + (sg % TT) * P
                            sc = scp.tile([P, 2 * S], F32, tag="sc",
                                          name="sc")
                            nc.tensor.matmul(
                                sc[:, 0:S], kst[0:HD, kc:kc + P],
                                qst[tch][0:HD, 0:S],
                                start=True, stop=True, tile_position=(0, 0))
                            nc.tensor.matmul(
                                sc[:, S:2 * S], kst[HD:P, kc:kc + P],
                                qst[tch][HD:P, 0:S],
                                start=True, stop=True, tile_position=(64, 0))
                            p_t = pp.tile([P, 2 * S], BF16, tag="p",
                                          name="p_t")
                            if si // TT == tcl:
                                c0 = (si % TT) * P
                                mk = masks[si % TT]
                                we = c0 + P
                                nc.vector.tensor_add(
                                    out=sc[:, 0:we], in0=sc[:, 0:we],
                                    in1=mk[:, 0:we])
                                nc.vector.tensor_add(
                                    out=sc[:, S:S + we], in0=sc[:, S:S + we],
                                    in1=mk[:, 0:we])
                                if c0:
                                    nc.gpsimd.memset(p_t[:, 0:c0], 0.0)
                                nc.scalar.activation(
                                    out=p_t[:, c0:], in_=sc[:, c0:],
                                    func=AF.Exp, scale=SCALE)
                            else:
                                nc.scalar.activation(
                                    out=p_t[:], in_=sc[:],
                                    func=AF.Exp, scale=SCALE)
                            first, last = si == 0, si == n_s - 1
                            nc.tensor.matmul(o_psa[:], v65[sg][:, 0:65],
                                             p_t[:, 0:S],
                                             start=first, stop=last)
                            nc.tensor.matmul(o_psb[:], v65[sg][:, 65:130],
                                             p_t[:, S:2 * S],
                                             start=first, stop=last)
                        # normalize: l sits in row 64 of each o_ps
                        la = l2p.tile([1, S], F32, tag="la", name="la")
                        lb = l2p.tile([1, S], F32, tag="lb", name="lb")
                        nc.vector.tensor_copy(la[:], o_psa[HD:HD + 1, :])
                        nc.vector.tensor_copy(lb[:], o_psb[HD:HD + 1, :])
                        linva = l2p.tile([1, S], BF16, tag="lia", name="linva")
                        linvb = l2p.tile([1, S], BF16, tag="lib", name="linvb")
                        with nc.allow_low_precision(
                                reason="1/l fits bf16; l in [0.5, 4e3]"):
                            nc.vector.reciprocal(out=linva[:], in_=la[:])
                            nc.vector.reciprocal(out=linvb[:], in_=lb[:])
                        lbc = lbcp.tile([P, S], F32, tag="lbc", name="lbc")
                        nc.tensor.matmul(lbc[:], C["selA"][:], linva[:],
                                         start=True, stop=False)
                        nc.tensor.matmul(lbc[:], C["selB"][:], linvb[:],
                                         start=False, stop=True)
                        lbc_sb = l2p.tile([P, S], BF16, tag="lbcs",
                                          name="lbc_sb")
                        nc.vector.tensor_copy(lbc_sb[:], lbc[:])
                        o_n = onp.tile([P, S], BF16, tag="on", name="o_n")
                        nc.vector.tensor_mul(out=o_n[0:HD, :],
                                             in0=o_psa[0:HD, :],
                                             in1=lbc_sb[0:HD, :])
                        nc.vector.tensor_mul(out=o_n[HD:P, :],
                                             in0=o_psb[0:HD, :],
                                             in1=lbc_sb[HD:P, :])
                        nc.sync.dma_start(
                            out=a2ao_in[tch * P:(tch + 1) * P, :],
                            in_=o_n[:])

        with nc.named_scope(f"a2ao_{ci}"):
            if stub_cc:
                nc.sync.dma_start(out=a2ao_out[:, :], in_=a2ao_in[:, :])
            else:
                nc.gpsimd.collective_compute(
                    "AllToAll", ALU.bypass,
                    replica_groups=[list(range(N_CORES))],
                    ins=[a2ao_in.opt()], outs=[a2ao_out.opt()],
                )

        # ================= Wo projection + residual =================
        with nc.named_scope(f"wo_{ci}"), \
             tc.tile_pool(name="wosb", bufs=1) as wosb, \
             tc.tile_pool(name="wops", bufs=1, space="PSUM") as wops:
            wo_sb = wosb.tile([P, KT * D], BF16, tag="wo", name="wo_sb")
            nc.scalar.dma_start(
                out=wo_sb[:].rearrange("p (k f) -> p k f", k=KT),
                in_=wo.ap().rearrange("(k p) f -> p k f", p=P))
            ao_big = wosb.tile([P, KT * S], BF16, tag="ao", name="ao_big")
            for kb in range(2):
                nc.sync.dma_start(
                    out=ao_big[:, kb * 4 * S:(kb + 1) * 4 * S]
                        .rearrange("p (k s) -> p k s", k=4),
                    in_=a2ao_out[kb * 4 * P:(kb + 1) * 4 * P, :]
                        .rearrange("(k p) s -> p k s", p=P))
            pso = [wops.tile([P, S], F32, tag=f"wo{i}", name=f"wo{i}")
                   for i in range(8)]
            for kt in range(KT):
                for tt in range(TT):
                    for dc in range(2):
                        nc.tensor.matmul(
                            pso[tt * 2 + dc][:],
                            ao_big[:, kt * S + tt * P:kt * S + (tt + 1) * P],
                            wo_sb[:, kt * D + dc * 512:kt * D + (dc + 1) * 512],
                            start=(kt == 0), stop=False)
            for tt in range(TT):
                for dc in range(2):
                    nc.tensor.matmul(
                        pso[tt * 2 + dc][:], C["ones1"][:],
                        C["bor"][:, dc * 512:(dc + 1) * 512],
                        start=False, stop=True)
                    c = tt * D + dc * 512
                    nc.vector.tensor_add(out=r1_big[:, c:c + 512],
                                         in0=pso[tt * 2 + dc][:],
                                         in1=x_big[:, c:c + 512])

        # ================= LN2 + FFN =================
        h2T = [persist.tile([P, S], BF16, tag=f"h2T{k}", name=f"h2T{k}")
               for k in range(KT)]
        with nc.named_scope(f"ln2_{ci}"), \
             tc.tile_pool(name="h2p", bufs=1) as h2p:
            h2 = _layernorm(nc, tc, C, r1_big[:], h2p, f"h2_{ci}")
            with tc.tile_pool(name="tr2p", bufs=4, space="PSUM") as tr2p:
                for kt in range(KT):
                    for i in range(TT):
                        pt2 = tr2p.tile([P, P], BF16, tag="tr2", name="pt2")
                        nc.tensor.transpose(
                            pt2[:], h2[i][:, kt * P:(kt + 1) * P], ident[:])
                        nc.vector.tensor_scalar(
                            out=h2T[kt][:, i * P:(i + 1) * P], in0=pt2[:],
                            scalar1=C["g2"][:, kt:kt + 1],
                            scalar2=C["b2l"][:, kt:kt + 1],
                            op0=ALU.mult, op1=ALU.add,
                        )

        with tc.tile_pool(name="ff1sb", bufs=1) as ff1sb:
            ff1 = [ff1sb.tile([P, S], BF16, tag=f"ff1_{k}",
                              name=f"ff1_{k}") for k in range(FT)]
            with nc.named_scope(f"ffn1_{ci}"), \
                 tc.tile_pool(name="w1s", bufs=2) as w1s, \
                 tc.tile_pool(name="ff1p", bufs=4, space="PSUM") as ff1p:
                for q in range(4):
                    w1q = w1s.tile([P, KT * D], BF16, tag="w1q", name="w1q")
                    nc.scalar.dma_start(
                        out=w1q[:].rearrange("p (k f) -> p k f", k=KT),
                        in_=w1[:, q * D:(q + 1) * D]
                            .rearrange("(k p) f -> p k f", p=P))
                    for fl in range(KT):
                        ft = q * KT + fl
                        ps = ff1p.tile([P, S], F32, tag="ff1", name="ps")
                        for kt in range(KT):
                            nc.tensor.matmul(
                                ps[:], w1q[:, kt * D + fl * P:
                                           kt * D + (fl + 1) * P],
                                h2T[kt][:],
                                start=(kt == 0), stop=(kt == KT - 1))
                        nc.vector.tensor_scalar(
                            out=ff1[ft][:], in0=ps[:],
                            scalar1=C["b1s"][:, ft:ft + 1], scalar2=0.0,
                            op0=ALU.add, op1=ALU.max)

            with nc.named_scope(f"ffn2_{ci}"), \
                 tc.tile_pool(name="w2s", bufs=2) as w2s, \
                 tc.tile_pool(name="yp", bufs=1) as yp, \
                 tc.tile_pool(name="ff2p", bufs=1, space="PSUM") as ff2p:
                y_big = yp.tile([P, TT * D], F32, tag="y_big", name="y_big")
                ps2 = [ff2p.tile([P, S], F32, tag=f"ff2_{i}",
                                 name=f"ff2_{i}") for i in range(8)]
                for q in range(4):
                    w2q = w2s.tile([P, KT * D], BF16, tag="w2q", name="w2q")
                    nc.scalar.dma_start(
                        out=w2q[:].rearrange("p (k f) -> p k f", k=KT),
                        in_=w2[q * KT * P:(q + 1) * KT * P, :]
                            .rearrange("(k p) f -> p k f", p=P))
                    for kl in range(KT):
                        kt = q * KT + kl
                        for tt in range(TT):
                            for dc in range(2):
                                nc.tensor.matmul(
                                    ps2[tt * 2 + dc][:],
                                    ff1[kt][:, tt * P:(tt + 1) * P],
                                    w2q[:, kl * D + dc * 512:
                                        kl * D + (dc + 1) * 512],
                                    start=(kt == 0), stop=False)
                for tt in range(TT):
                    for dc in range(2):
                        nc.tensor.matmul(
                            ps2[tt * 2 + dc][:], C["ones1"][:],
                            C["b2r"][:, dc * 512:(dc + 1) * 512],
                            start=False, stop=True)
                        c = tt * D + dc * 512
                        nc.vector.tensor_add(out=y_big[:, c:c + 512],
                                             in0=ps2[tt * 2 + dc][:],
                                             in1=r1_big[:, c:c + 512])
                yr = y[:, :].rearrange("(t p) d -> p t d", p=P)
                for half in range(2):
                    nc.sync.dma_start(
                        out=yr[:, half * 2:(half + 1) * 2, :],
                        in_=y_big[:, half * 2 * D:(half + 1) * 2 * D]
                            .rearrange("p (t d) -> p t d", t=2))


def _shard_inputs(inputs):
    import ml_dtypes
    bf = ml_dtypes.bfloat16
    x = np.ascontiguousarray(
        np.asarray(inputs["x"], np.float32).reshape(NTOK, D))
    com = dict(
        wq=np.ascontiguousarray(np.asarray(inputs["Wq"], np.float32)
                                .transpose(1, 0, 2).reshape(D, D)
                                .astype(bf)),
        wk=np.ascontiguousarray(np.asarray(inputs["Wk"], np.float32)
                                .transpose(1, 0, 2).reshape(D, D)
                                .astype(bf)),
        wv=np.ascontiguousarray(np.asarray(inputs["Wv"], np.float32)
                                .transpose(1, 0, 2).reshape(D, D)
                                .astype(bf)),
        wo=np.ascontiguousarray(np.asarray(inputs["Wo"], np.float32)
                                .astype(bf)),
        w1=np.ascontiguousarray(np.asarray(inputs["W1"], np.float32)
                                .astype(bf)),
        w2=np.ascontiguousarray(np.asarray(inputs["W2"], np.float32)
                                .astype(bf)),
        bo=np.asarray(inputs["bo"], np.float32).astype(bf),
        b1=np.asarray(inputs["b1"], np.float32),
        b2=np.asarray(inputs["b2"], np.float32).astype(bf),
        ln1_g=np.asarray(inputs["ln1_g"], np.float32),
        ln1_b=np.asarray(inputs["ln1_b"], np.float32),
        ln2_g=np.asarray(inputs["ln2_g"], np.float32),
        ln2_b=np.asarray(inputs["ln2_b"], np.float32),
    )
    maps = []
    for c in range(N_CORES):
        m = dict(com)
        m["x"] = x[c * S:(c + 1) * S]
        maps.append(m)
    return maps


def _get_nc():
    if "nc" not in _CACHE:
        _CACHE["nc"] = _build()
    return _CACHE["nc"]


def _run(in_maps):
    from concourse.bass_utils import run_bass_kernel_spmd
    nc = _get_nc()
    res = run_bass_kernel_spmd(nc, in_maps, core_ids=list(range(N_CORES)))
    return res.results


def kernel(**inputs):
    in_maps = _shard_inputs(inputs)
    results = _run(in_maps)
    out = np.concatenate([results[c]["y"] for c in range(N_CORES)], axis=0)
    return out.reshape(B, T, D)
